# revision 1
# baseline (speedup 1.0000x reference)
"""Trainium2 Bass kernel for a 2-layer GCN + global mean pool + FC.

Strategy (8 NeuronCores, SPMD single NEFF):
  - Nodes (and their in-edges) partitioned by dst across 8 cores; weights
    replicated; h1 shards AllGathered between layers; pooled sums AllReduced.
  - Per 128-edge chunk, h[src] rows are fetched with dma_gather (row i ->
    partition i%128) and scatter-added via a one-hot mask matmul on the
    TensorEngine: agg[128d,64f] += S[e,d].T @ msgs[e,f] accumulating in PSUM.
  - S masks are pure 0/1 one-hots in bf16, generated in one batched DVE
    tensor_tensor op per supergather set (iota vs dst_local broadcast APs);
    the edge norm (dinv_sqrt[src]*dinv_sqrt[dst]) is folded into the msgs
    during the f32->bf16 convert of each gather tile (one batched DVE op).
  - Self-loop terms use the core's own contiguous rows (sequential DMA) and
    are fused into the per-block epilogue -- no per-edge gathers for them.
  - dma_gather indices are int16 (max 32767), so nodes are split into two
    sets A/B by their position within the owner's shard (local offset < 3200);
    gather sources are the correspondingly reordered xA/xB (host-permuted)
    and h1fullA/h1fullB. The A half of the h1 AllGather is issued as soon as
    the first 25 blocks are done, overlapping the rest of layer 1.
"""

import numpy as np
import ml_dtypes

from concourse import bacc, bass, mybir, bass_utils
from concourse.masks import make_identity
import concourse.tile as tile

N = 50000
E = 800000
F = 64          # feature width of x / h1 / h2
G = 128         # number of graphs
OUT = 8
P = 128
C = 8
NSH = N // C    # 6250 nodes per core
ABL = 3200      # A/B split point (local offset, 25 blocks)
NA = C * ABL            # rows in the A gather source (25600)
NBB = C * (NSH - ABL)   # rows in the B gather source (24400)
NB = (NSH + P - 1) // P   # 49 dst blocks per core
ABLK = ABL // P           # 25 blocks in A
SBLK = 4                  # dst blocks per supergather
NSB = (NB + SBLK - 1) // SBLK
F32 = mybir.dt.float32
BF16 = mybir.dt.bfloat16
I16 = mybir.dt.int16


def _bcast_ap(ap, dims):
    """Build a broadcast view of `ap` with explicit [step, count] dims."""
    return bass.AP(tensor=ap.tensor, offset=ap.offset, ap=dims)


def _ab_index(n):
    """Map global node id -> (set, idx-within-set) for the A/B split."""
    r, l = n // NSH, n % NSH
    s = l >= ABL
    return s, np.where(s, r * (NSH - ABL) + (l - ABL), r * ABL + l)


def _preprocess(src, dst, batch):
    """Host-side index preprocessing (pure integer/index work)."""
    src = np.asarray(src).astype(np.int64)
    dst = np.asarray(dst).astype(np.int64)
    batch = np.asarray(batch).astype(np.int64)

    deg = np.bincount(dst, minlength=N).astype(np.float32) + 1.0
    dinv = (1.0 / np.sqrt(deg)).astype(np.float32)
    norm_all = (dinv[src] * dinv[dst]).astype(np.float32)
    st_all, sidx_all = _ab_index(src)
    st_all = st_all.astype(np.int64)

    core_groups = []
    counts = np.zeros((C, NB, 2), np.int64)
    for c in range(C):
        lo = c * NSH
        m = (dst >= lo) & (dst < lo + NSH)
        es, ed, en = sidx_all[m], dst[m], norm_all[m]
        st = st_all[m]
        dloc = ed - lo
        blk = dloc >> 7
        sb = blk // SBLK
        blkin = blk - sb * SBLK
        key = (sb * 2 + st) * SBLK + blkin
        order = np.argsort(key, kind="stable")
        es, en, dloc, key = es[order], en[order], dloc[order], key[order]
        np.add.at(counts[c], (blk[order], st[order]), 1)
        core_groups.append((es, en, dloc, key))

    nch_bs = np.ceil(counts.max(axis=0) / P).astype(np.int64)  # [NB, 2]
    nch_bs = np.maximum(nch_bs, 1)

    nch_sb = np.zeros((NSB, 2), np.int64)
    for b in range(NB):
        nch_sb[b // SBLK] += nch_bs[b]
    chunk_base = {}
    idxcol_base = {}
    tot_chunks = 0
    idx_cols = [0, 0]
    for sbi in range(NSB):
        for s in range(2):
            chunk_base[(sbi, s)] = tot_chunks
            tot_chunks += int(nch_sb[sbi, s])
            idxcol_base[(sbi, s)] = idx_cols[s]
            idx_cols[s] += int(nch_sb[sbi, s]) * (P // 16)
    off_in_tile = np.zeros((NB, 2), np.int64)
    for sbi in range(NSB):
        run = [0, 0]
        for b in range(sbi * SBLK, min((sbi + 1) * SBLK, NB)):
            for s in range(2):
                off_in_tile[b, s] = run[s]
                run[s] += int(nch_bs[b, s])

    plan = dict(nch_bs=nch_bs, nch_sb=nch_sb, chunk_base=chunk_base,
                idxcol_base=idxcol_base, off_in_tile=off_in_tile,
                tot_chunks=tot_chunks, idx_cols=idx_cols)

    per_core = []
    for c in range(C):
        es, en, dloc, key = core_groups[c]
        bounds = np.searchsorted(key, np.arange(NSB * 2 * SBLK + 1))
        idx_parts = [[], []]
        dl_parts = []
        nm_parts = []
        for sbi in range(NSB):
            for s in range(2):
                for b in range(sbi * SBLK, min((sbi + 1) * SBLK, NB)):
                    k = (sbi * 2 + s) * SBLK + (b - sbi * SBLK)
                    g0, g1 = bounds[k], bounds[k + 1]
                    n = g1 - g0
                    want = int(nch_bs[b, s]) * P
                    assert n <= want
                    gi = np.zeros(want, np.int64)
                    gd = np.zeros(want, np.int64)
                    gn = np.zeros(want, np.float32)
                    gi[:n] = es[g0:g1]
                    gd[:n] = dloc[g0:g1] - (b << 7)
                    gn[:n] = en[g0:g1]
                    idx_parts[s].append(gi)
                    dl_parts.append(gd)
                    nm_parts.append(gn)
        dstloc = np.concatenate(dl_parts).reshape(-1, P).T
        normv = np.concatenate(nm_parts).reshape(-1, P).T.astype(np.float32)
        idx = []
        for s in range(2):
            stk = np.concatenate(idx_parts[s]).astype(np.int16)
            idx.append(np.tile(stk.reshape(-1, 16).T, (8, 1)))
        batchloc = np.full((P, NB), -1.0, np.float32)
        full = np.full(NB * P, -1.0, np.float32)
        full[:NSH] = batch[c * NSH:(c + 1) * NSH]
        batchloc[:, :] = full.reshape(NB, P).T
        selfw = np.zeros(NB * P, np.float32)
        selfw[:NSH] = 1.0 / deg[c * NSH:(c + 1) * NSH]
        selfw = selfw.reshape(NB, P).T.copy()
        per_core.append(dict(
            idx0=idx[0], idx1=idx[1],
            dstloc=dstloc.astype(ml_dtypes.bfloat16),
            normv=normv, batchloc=batchloc.astype(ml_dtypes.bfloat16), selfw=selfw))

    cnt = np.bincount(batch, minlength=G).astype(np.float32)
    invc = (1.0 / np.maximum(cnt, 1.0)).astype(np.float32)
    return plan, per_core, invc


def _build(plan):
    """Build the SPMD Bass program (identical for all cores)."""
    nch_bs = plan["nch_bs"]
    nch_sb = plan["nch_sb"]
    chunk_base = plan["chunk_base"]
    idxcol_base = plan["idxcol_base"]
    off_in_tile = plan["off_in_tile"]
    NCH = plan["tot_chunks"]
    icols = plan["idx_cols"]

    nc = bacc.Bacc("TRN2", target_bir_lowering=False, debug=False,
                   num_devices=C, num_swdge_queues=4)

    xA = nc.dram_tensor("xA", [NA, F], F32, kind="ExternalInput")
    xB = nc.dram_tensor("xB", [NBB, F], F32, kind="ExternalInput")
    xown = nc.dram_tensor("xown", [NSH, F], F32, kind="ExternalInput")
    idx0 = nc.dram_tensor("idx0", [P, icols[0]], I16, kind="ExternalInput")
    idx1 = nc.dram_tensor("idx1", [P, icols[1]], I16, kind="ExternalInput")
    dstloc = nc.dram_tensor("dstloc", [P, NCH], BF16, kind="ExternalInput")
    normv = nc.dram_tensor("normv", [P, NCH], F32, kind="ExternalInput")
    batchloc = nc.dram_tensor("batchloc", [P, NB], BF16, kind="ExternalInput")
    selfw_in = nc.dram_tensor("selfw", [P, NB], F32, kind="ExternalInput")
    iota_in = nc.dram_tensor("iota", [P, P], BF16, kind="ExternalInput")
    W1 = nc.dram_tensor("W1", [F, F], F32, kind="ExternalInput")
    W2 = nc.dram_tensor("W2", [F, F], F32, kind="ExternalInput")
    Wfc = nc.dram_tensor("Wfc", [F, OUT], F32, kind="ExternalInput")
    b1b = nc.dram_tensor("b1b", [P, F], F32, kind="ExternalInput")
    b2b = nc.dram_tensor("b2b", [P, F], F32, kind="ExternalInput")
    bfcb = nc.dram_tensor("bfcb", [P, OUT], F32, kind="ExternalInput")
    invc_in = nc.dram_tensor("invc", [F, G], F32, kind="ExternalInput")
    out = nc.dram_tensor("out", [G, OUT], F32, kind="ExternalOutput")

    gq = [0]  # rotating swdge queue counter

    with tile.TileContext(nc) as tc:
        with (
            tc.tile_pool(name="const", bufs=1) as cp,
            tc.tile_pool(name="gpool", bufs=2) as gp,
            tc.tile_pool(name="mpool", bufs=3) as mp,
            tc.tile_pool(name="spool", bufs=3) as sp,
            tc.tile_pool(name="epool", bufs=3) as ep,
            tc.tile_pool(name="psA", bufs=2, space="PSUM") as psA,
            tc.tile_pool(name="psB", bufs=1, space="PSUM") as psB,
            tc.tile_pool(name="dram", bufs=1, space="DRAM") as dram,
        ):
            # ---- constants / metadata loads ----
            iota_sb = cp.tile([P, P], BF16, tag="iota")
            nc.sync.dma_start(iota_sb[:], iota_in[:])
            ident = cp.tile([P, P], F32, tag="ident")
            make_identity(nc, ident[:])
            idx_sb = [cp.tile([P, icols[0]], I16, tag="idx0", name="idx_sb0"),
                      cp.tile([P, icols[1]], I16, tag="idx1", name="idx_sb1")]
            nc.scalar.dma_start(idx_sb[0][:], idx0[:])
            nc.scalar.dma_start(idx_sb[1][:], idx1[:])
            dl_sb = cp.tile([P, NCH], BF16, tag="dstloc")
            nc.scalar.dma_start(dl_sb[:], dstloc[:])
            nm_sb = cp.tile([P, NCH], F32, tag="normv")
            nc.scalar.dma_start(nm_sb[:], normv[:])
            bl_sb = cp.tile([P, NB], BF16, tag="batchloc")
            nc.scalar.dma_start(bl_sb[:], batchloc[:])
            sw_sb = cp.tile([P, NB], F32, tag="selfw")
            nc.sync.dma_start(sw_sb[:], selfw_in[:])
            W1_sb = cp.tile([F, F], F32, tag="W1")
            nc.sync.dma_start(W1_sb[:], W1[:])
            W2_sb = cp.tile([F, F], F32, tag="W2")
            nc.sync.dma_start(W2_sb[:], W2[:])
            Wfc_sb = cp.tile([F, OUT], F32, tag="Wfc")
            nc.sync.dma_start(Wfc_sb[:], Wfc[:])
            b1_sb = cp.tile([P, F], F32, tag="b1b")
            nc.sync.dma_start(b1_sb[:], b1b[:])
            b2_sb = cp.tile([P, F], F32, tag="b2b")
            nc.sync.dma_start(b2_sb[:], b2b[:])
            bfc_sb = cp.tile([P, OUT], F32, tag="bfcb")
            nc.sync.dma_start(bfc_sb[:], bfcb[:])
            invc_sb = cp.tile([F, G], F32, tag="invc")
            nc.sync.dma_start(invc_sb[:], invc_in[:])

            h1shardA = dram.tile([ABL, 2 * F], BF16)
            h1shardB = dram.tile([NSH - ABL, 2 * F], BF16)
            h1fullA = dram.tile([NA, 2 * F], BF16, addr_space="Shared")
            h1fullB = dram.tile([NBB, 2 * F], BF16, addr_space="Shared")
            pool_in = dram.tile([F, G], F32)
            pool_out = dram.tile([F, G], F32, addr_space="Shared")

            pool_ps = psB.tile([F, G], F32, tag="pool")

            # batched pool one-hots for all 49 blocks (generated at startup)
            Sp_all = cp.tile([P, NB, G], BF16, tag="Sp_all")
            blm = bl_sb[:, :]
            nc.vector.tensor_tensor(
                out=Sp_all[:],
                in0=_bcast_ap(iota_sb[:], [iota_sb[:].ap[0], [0, NB], [1, G]]),
                in1=_bcast_ap(blm, [blm.ap[0], [blm.ap[1][0], NB], [0, G]]),
                op=mybir.AluOpType.is_equal,
            )

            def gather(t, src_ap, idx_tile, icol0, nidx, g_w):
                q = gq[0] % 4
                gq[0] += 1
                nc.gpsimd.dma_gather(
                    t[:], src_ap, idx_tile[:, icol0:icol0 + nidx // 16],
                    nidx, nidx, g_w,
                    single_packet=False, queue_num=q,
                )

            NBF = NB - 1          # full 128-row blocks in a shard
            LASTR = NSH - NBF * P  # rows in the last partial block

            def conv_layer(srcsAB, own_parts, W_sb, bb_sb, sink, h_dt,
                           g_dt=F32, g_w=F, own_dt=F32):
                # own rows for self-loop term: [128, NB, 64]
                x_own = ep.tile([P, NB, F], own_dt, tag="x_own", bufs=1)
                nc.vector.memset(x_own[:, NBF, :], 0.0)
                for (ap_src, b0, nrow) in own_parts:
                    nfull = nrow // P
                    if nfull:
                        nc.sync.dma_start(
                            x_own[:, b0:b0 + nfull, :],
                            ap_src[:nfull * P, :].rearrange("(b p) f -> p b f", p=P),
                        )
                    rem = nrow - nfull * P
                    if rem:
                        nc.sync.dma_start(
                            x_own[:rem, b0 + nfull, :],
                            ap_src[nfull * P:nrow, :],
                        )
                # batched self-loop term: tmp_all[:, b, :] = x_own[:, b, :]*selfw[:, b]
                tmp_all = ep.tile([P, NB, F], F32, tag="tmp_all", bufs=1)
                swm = sw_sb[:, :]
                nc.vector.tensor_tensor(
                    out=tmp_all[:],
                    in0=x_own[:],
                    in1=_bcast_ap(swm, [swm.ap[0], [swm.ap[1][0], NB], [0, F]]),
                    op=mybir.AluOpType.mult,
                )
                for sbi in range(NSB):
                    mt = {}
                    St = {}
                    for s in range(2):
                        nch = int(nch_sb[sbi, s])
                        if nch == 0:
                            continue
                        gt = gp.tile([P, nch, g_w], g_dt, tag=f"g{s}")
                        nidx = nch * P
                        gather(gt, srcsAB[s], idx_sb[s], idxcol_base[(sbi, s)],
                               nidx, g_w)
                        cb = chunk_base[(sbi, s)]
                        # fused norm-scale + f32->bf16 convert, one op per tile
                        m_t = mp.tile([P, nch, F], BF16, tag=f"m{s}")
                        nmap = nm_sb[:, cb:cb + nch]
                        nc.vector.tensor_tensor(
                            out=m_t[:],
                            in0=gt[:, :, 0:F],
                            in1=_bcast_ap(nmap, [nmap.ap[0], [nmap.ap[1][0], nch], [0, F]]),
                            op=mybir.AluOpType.mult,
                        )
                        mt[s] = m_t
                        # batched one-hot S for the whole supergather set
                        S_t = sp.tile([P, nch, P], BF16, tag=f"S{s}")
                        dmap = dl_sb[:, cb:cb + nch]
                        nc.vector.tensor_tensor(
                            out=S_t[:],
                            in0=_bcast_ap(iota_sb[:], [iota_sb[:].ap[0], [0, nch], [1, P]]),
                            in1=_bcast_ap(dmap, [dmap.ap[0], [dmap.ap[1][0], nch], [0, P]]),
                            op=mybir.AluOpType.is_equal,
                        )
                        St[s] = S_t
                    for b in range(sbi * SBLK, min((sbi + 1) * SBLK, NB)):
                        agg_ps = psA.tile([P, F], F32, tag="agg")
                        tot = int(nch_bs[b, 0] + nch_bs[b, 1])
                        done = 0
                        for s in range(2):
                            nch = int(nch_bs[b, s])
                            if nch == 0:
                                continue
                            off = int(off_in_tile[b, s])
                            for ci in range(nch):
                                nc.tensor.matmul(
                                    agg_ps[:], lhsT=St[s][:, off + ci, :],
                                    rhs=mt[s][:, off + ci, :],
                                    start=(done == 0), stop=(done == tot - 1),
                                )
                                done += 1
                        # epilogue: h = tanh((agg + selfw*own) @ W + b)
                        agg_sb = ep.tile([P, F], F32, tag="agg_sb", bufs=6)
                        nc.vector.tensor_add(agg_sb[:], agg_ps[:], tmp_all[:, b, :])
                        trp = psA.tile([F, P], F32, tag="tr")
                        nc.tensor.transpose(trp[:], agg_sb[:], ident[:])
                        aggT = ep.tile([F, P], F32, tag="aggT", bufs=6)
                        nc.vector.tensor_copy(aggT[:], trp[:])
                        h_ps = psA.tile([P, F], F32, tag="h")
                        nc.tensor.matmul(h_ps[:], lhsT=aggT[:], rhs=W_sb[:],
                                         start=True, stop=True)
                        hf_sb = ep.tile([P, F], F32, tag="hf_sb", bufs=6)
                        nc.vector.tensor_add(hf_sb[:], h_ps[:], bb_sb[:])
                        h_sb = ep.tile([P, F], h_dt, tag="h_sb", bufs=6)
                        nc.scalar.activation(h_sb[:], hf_sb[:],
                                             mybir.ActivationFunctionType.Tanh)
                        sink(b, h_sb)

            def sink1(b, h_sb):
                if b < ABLK:
                    r0 = b * P
                    nc.sync.dma_start(h1shardA[r0:r0 + P, 0:F], h_sb[:])
                else:
                    r0 = (b - ABLK) * P
                    rows = min(P, (NSH - ABL) - r0)
                    nc.sync.dma_start(h1shardB[r0:r0 + rows, 0:F], h_sb[:rows, :])

            def sink2(b, h_sb):
                nc.tensor.matmul(pool_ps[:], lhsT=h_sb[:], rhs=Sp_all[:, b, :],
                                 start=(b == 0), stop=(b == NB - 1),
                                 skip_group_check=True)

            conv_layer((xA[:], xB[:]), [(xown[:], 0, NSH)], W1_sb, b1_sb,
                       sink1, BF16)
            nc.gpsimd.collective_compute(
                "AllGather", mybir.AluOpType.bypass,
                ins=[h1shardA.opt()], outs=[h1fullA.opt()],
                replica_groups=[list(range(C))],
            )
            nc.gpsimd.collective_compute(
                "AllGather", mybir.AluOpType.bypass,
                ins=[h1shardB.opt()], outs=[h1fullB.opt()],
                replica_groups=[list(range(C))],
            )
            conv_layer((h1fullA[:], h1fullB[:]),
                       [(h1shardA[:, 0:F], 0, ABL),
                        (h1shardB[:, 0:F], ABLK, NSH - ABL)],
                       W2_sb, b2_sb, sink2, BF16,
                       g_dt=BF16, g_w=2 * F, own_dt=BF16)

            # ---- pooled tail ----
            poolT = ep.tile([F, G], F32, tag="poolT")
            nc.vector.tensor_copy(poolT[:], pool_ps[:])
            nc.sync.dma_start(pool_in[:], poolT[:])
            nc.gpsimd.collective_compute(
                "AllReduce", mybir.AluOpType.add,
                ins=[pool_in.opt()], outs=[pool_out.opt()],
                replica_groups=[list(range(C))],
            )
            poolR = ep.tile([F, G], F32, tag="poolR")
            nc.sync.dma_start(poolR[:], pool_out[:])
            nc.vector.tensor_mul(poolR[:], poolR[:], invc_sb[:])
            fc_ps = psB.tile([G, OUT], F32, tag="fc")
            nc.tensor.matmul(fc_ps[:], lhsT=poolR[:], rhs=Wfc_sb[:],
                             start=True, stop=True)
            out_sb = ep.tile([G, OUT], F32, tag="out_sb")
            nc.vector.tensor_add(out_sb[:], fc_ps[:], bfc_sb[:])
            nc.sync.dma_start(out[:], out_sb[:])

    nc.compile()
    return nc


def _in_maps(plan, per_core, invc, x, W1, b1, W2, b2, Wfc, bfc):
    iota = np.tile(np.arange(P, dtype=np.float32), (P, 1)).astype(ml_dtypes.bfloat16)
    xf = np.ascontiguousarray(np.asarray(x, np.float32))
    xr = xf.reshape(C, NSH, F)
    xA = np.ascontiguousarray(xr[:, :ABL, :].reshape(NA, F))
    xB = np.ascontiguousarray(xr[:, ABL:, :].reshape(NBB, F))
    shared = dict(
        xA=xA, xB=xB,
        iota=iota,
        W1=np.ascontiguousarray(np.asarray(W1, np.float32)),
        W2=np.ascontiguousarray(np.asarray(W2, np.float32)),
        Wfc=np.ascontiguousarray(np.asarray(Wfc, np.float32)),
        b1b=np.tile(np.asarray(b1, np.float32), (P, 1)),
        b2b=np.tile(np.asarray(b2, np.float32), (P, 1)),
        bfcb=np.tile(np.asarray(bfc, np.float32), (P, 1)),
        invc=np.tile(invc, (F, 1)),
    )
    maps = []
    for c in range(C):
        m = dict(shared)
        m.update(per_core[c])
        m["xown"] = xf[c * NSH:(c + 1) * NSH]
        maps.append({k: np.ascontiguousarray(v) for k, v in m.items()})
    return maps


_RUN_KWARGS = {}


def kernel(x, src, dst, batch, W1, b1, W2, b2, Wfc, bfc):
    plan, per_core, invc = _preprocess(src, dst, batch)
    nc = _build(plan)
    maps = _in_maps(plan, per_core, invc, x, W1, b1, W2, b2, Wfc, bfc)
    res = bass_utils.run_bass_kernel_spmd(
        nc, maps, core_ids=list(range(C)), **_RUN_KWARGS
    )
    kernel.last_results = res
    return np.asarray(res.results[0]["out"], np.float32)



# revision 4
# speedup vs baseline: 1.1024x; 1.1024x over previous
"""Trainium2 Bass kernel for a 2-layer GCN + global mean pool + FC.

v2 strategy (8 NeuronCores, SPMD single NEFF):
  - Nodes (and in-edges) partitioned by dst across 8 cores.
  - Layer 1 messages are host-expanded into a contiguous per-edge stream
    (x~ = dinv*x rows in dst-chunk order, bf16) -- no gathers, no Q7 work.
  - Layer 2 gathers raw bf16 h~1 rows with dma_gather from a 256B-padded
    table (h1pad[N,128]); indices sorted ascending per call; A/B split at
    row 32768 for int16 indices.
  - S one-hot masks are built per supergather set by a linear PE matmul
    D[p,(c,d)] = iota[d] - dstloc[p,c]  (exact small ints in bf16)
    into PSUM, then a single DVE tensor_scalar is_equal-vs-0 pass
    (PSUM source => does not take the DVE/GpSimd shared SBUF port, so
    Q7 descriptor generation never stalls).
  - Normalization is factorized: norm_e = dinv[src]*dinv[dst].  dinv[src]
    is folded into the gathered/streamed rows (x~, h~1); dinv[dst] is the
    per-partition ACT scale of the epilogue tanh.  Self-loops are diagonal
    S chunks over the core's own rows.  Bias enters via a rank-1
    sqrtdeg (x) b matmul (cancels the dinv scale).
  - agg is accumulated transposed: aggT[64f,128d] += tile^T @ S, so the
    epilogue is a direct matmul with W -- no transposes anywhere.
  - AllGather of h~1 split in two halves (first issued mid-layer-1), then
    HWDGE row-strided expand into h1pad.
"""

import numpy as np
import ml_dtypes

from concourse import bacc, bass, mybir, bass_utils
import concourse.tile as tile

N = 50000
E = 800000
F = 64
G = 128
OUT = 8
P = 128
C = 8
NSH = N // C          # 6250 nodes per core
NB = (NSH + P - 1) // P   # 49 dst blocks per core
SBLK = 4
NSET = (NB + SBLK - 1) // SBLK  # 13 sets
ASPLIT = 32768        # L2 gather A/B split (int16 index limit)
ABLOCKS = 25          # shard-A blocks (AG split): rows 0..3199
SA_ROWS = ABLOCKS * P         # 3200
SB_ROWS = NSH - SA_ROWS       # 3050
PADV = 200.0          # dstloc value for pad slots (never matches iota)
F32 = mybir.dt.float32
BF16 = mybir.dt.bfloat16
I16 = mybir.dt.int16
BF = ml_dtypes.bfloat16


def _bcast_ap(ap, dims):
    return bass.AP(tensor=ap.tensor, offset=ap.offset, ap=dims)


def _set_blocks(s):
    return list(range(s * SBLK, min((s + 1) * SBLK, NB)))


def _preprocess(x, src, dst, batch):
    """Host-side planning: pure index/structure work + layout transforms."""
    src = np.asarray(src).astype(np.int64)
    dst = np.asarray(dst).astype(np.int64)
    batch = np.asarray(batch).astype(np.int64)
    x = np.asarray(x, np.float32)

    deg = np.bincount(dst, minlength=N).astype(np.float64) + 1.0
    dinv = (1.0 / np.sqrt(deg)).astype(np.float32)
    xt = (x * dinv[:, None]).astype(BF)      # x~ = dinv * x (node-level scale)

    # per-core, per-block edge lists sorted by src
    core_e = []          # [c][b] -> sorted src array (global ids)
    for c in range(C):
        lo = c * NSH
        m = (dst >= lo) & (dst < lo + NSH)
        es, ed = src[m], dst[m] - lo
        blk = ed >> 7
        dl = ed & 127
        order = np.lexsort((es, blk))
        es, dl, blk = es[order], dl[order], blk[order]
        bounds = np.searchsorted(blk, np.arange(NB + 1))
        per_b = []
        for b in range(NB):
            g0, g1 = bounds[b], bounds[b + 1]
            per_b.append((es[g0:g1], dl[g0:g1]))
        core_e.append(per_b)

    # common (max-over-core) chunk counts
    cnt1 = np.zeros((C, NB), np.int64)
    cntA = np.zeros((C, NB), np.int64)
    cntB = np.zeros((C, NB), np.int64)
    for c in range(C):
        for b in range(NB):
            es, _ = core_e[c][b]
            cnt1[c, b] = len(es)
            na = int((es < ASPLIT).sum())
            cntA[c, b] = na
            cntB[c, b] = len(es) - na
    nch1 = np.maximum(np.ceil(cnt1.max(axis=0) / P).astype(np.int64), 1)
    nchA = np.maximum(np.ceil(cntA.max(axis=0) / P).astype(np.int64), 1)
    nchB = np.maximum(np.ceil(cntB.max(axis=0) / P).astype(np.int64), 1)

    # per-set mm counts
    n1_set = np.array([sum(nch1[b] + 1 for b in _set_blocks(s))
                       for s in range(NSET)])
    n2_set = np.array([sum(nchA[b] + nchB[b] + 1 for b in _set_blocks(s))
                       for s in range(NSET)])
    NM1 = int(n1_set.max()) + 1
    NM2 = int(n2_set.max()) + 1
    NMX = max(NM1, NM2)
    NCH1 = int(n1_set.sum())

    # Dconst [NMX, (NMX-1)*P]: row0 = iota tiled; row k = -1 on chunk k-1
    dconst = np.zeros((NMX, (NMX - 1) * P), np.float32)
    dconst[0] = np.tile(np.arange(P, dtype=np.float32), NMX - 1)
    for k in range(1, NMX):
        dconst[k, (k - 1) * P:k * P] = -1.0

    # gather call column bases (shared across cores)
    nA_set = np.array([sum(nchA[b] for b in _set_blocks(s)) for s in range(NSET)])
    nB_set = np.array([sum(nchB[b] for b in _set_blocks(s)) for s in range(NSET)])
    icolsA = int(nA_set.sum()) * (P // 16)
    icolsB = int(nB_set.sum()) * (P // 16)

    # mm schedule per set (shared): list of (block, kind, tile_col)
    # kind: 0=self(own), 1=A-gather, 2=B-gather / L1: 3=stream
    sched2 = []
    for s in range(NSET):
        lst = []
        ao = bo = 0
        for b in _set_blocks(s):
            lst.append((b, 0, 0))
            for i in range(int(nchA[b])):
                lst.append((b, 1, ao)); ao += 1
            for i in range(int(nchB[b])):
                lst.append((b, 2, bo)); bo += 1
        sched2.append(lst)
    sched1 = []
    for s in range(NSET):
        lst = []
        co = 0
        for b in _set_blocks(s):
            for i in range(int(nch1[b])):
                lst.append((b, 3, co)); co += 1
            lst.append((b, 3, co)); co += 1   # self chunk (in-stream)
        sched1.append(lst)

    plan = dict(nch1=nch1, nchA=nchA, nchB=nchB, n1_set=n1_set, n2_set=n2_set,
                nA_set=nA_set, nB_set=nB_set, NM1=NM1, NM2=NM2, NMX=NMX,
                NCH1=NCH1, icolsA=icolsA, icolsB=icolsB,
                sched1=sched1, sched2=sched2)

    # ---- per-core tensors ----
    per_core = []
    for c in range(C):
        xs = np.zeros((P, NCH1, F), BF)
        d1 = np.zeros((NM1, NSET * P), np.float32)
        d2 = np.zeros((NM2, NSET * P), np.float32)
        d1[0] = 1.0
        d2[0] = 1.0
        idxA_parts, idxB_parts = [], []
        ch1 = 0
        for s in range(NSET):
            k1 = 0
            k2 = 0
            dcol = slice(s * P, (s + 1) * P)
            # L1: stream chunks + dstloc
            for b in _set_blocks(s):
                es, dl = core_e[c][b]
                for i in range(int(nch1[b])):
                    rows = es[i * P:(i + 1) * P]
                    dls = dl[i * P:(i + 1) * P]
                    nr = len(rows)
                    col = np.full(P, PADV, np.float32)
                    col[:nr] = dls
                    if nr:
                        xs[:nr, ch1, :] = xt[rows]
                    d1[1 + k1, dcol] = col
                    k1 += 1
                    ch1 += 1
                # self chunk
                nr = min(P, NSH - b * P)
                col = np.full(P, PADV, np.float32)
                col[:nr] = np.arange(nr, dtype=np.float32)
                xs[:nr, ch1, :] = xt[c * NSH + b * P: c * NSH + b * P + nr]
                d1[1 + k1, dcol] = col
                k1 += 1
                ch1 += 1
            # L2: self first, then A chunks, then B chunks (block-major)
            for b in _set_blocks(s):
                nr = min(P, NSH - b * P)
                col = np.full(P, PADV, np.float32)
                col[:nr] = np.arange(nr, dtype=np.float32)
                d2[1 + k2, dcol] = col
                k2 += 1
                es, dl = core_e[c][b]
                ma = es < ASPLIT
                esA, dlA = es[ma], dl[ma]
                esB, dlB = es[~ma] - ASPLIT, dl[~ma]
                for i in range(int(nchA[b])):
                    rows = esA[i * P:(i + 1) * P]
                    dls = dlA[i * P:(i + 1) * P]
                    nr = len(rows)
                    col = np.full(P, PADV, np.float32)
                    col[:nr] = dls
                    gi = np.zeros(P, np.int64)
                    gi[:nr] = rows
                    idxA_parts.append(gi)
                    d2[1 + k2, dcol] = col
                    k2 += 1
                for i in range(int(nchB[b])):
                    rows = esB[i * P:(i + 1) * P]
                    dls = dlB[i * P:(i + 1) * P]
                    nr = len(rows)
                    col = np.full(P, PADV, np.float32)
                    col[:nr] = dls
                    gi = np.zeros(P, np.int64)
                    gi[:nr] = rows
                    idxB_parts.append(gi)
                    d2[1 + k2, dcol] = col
                    k2 += 1
            assert k1 == n1_set[s] and k2 == n2_set[s]

        def mk_idx(parts):
            if not parts:
                return np.zeros((P, 0), np.int16)
            stk = np.concatenate(parts).astype(np.int16)
            return np.tile(stk.reshape(-1, 16).T, (8, 1))

        sq = np.ones(NB * P, np.float32)
        dv = np.ones(NB * P, np.float32)
        bl = np.full(NB * P, -1.0, np.float32)
        own = np.arange(NSH)
        sq[:NSH] = np.sqrt(deg[c * NSH + own]).astype(np.float32)
        dv[:NSH] = dinv[c * NSH + own]
        bl[:NSH] = batch[c * NSH + own]
        per_core.append(dict(
            xs=np.ascontiguousarray(xs.reshape(P, NCH1 * F)),
            idxA=mk_idx(idxA_parts), idxB=mk_idx(idxB_parts),
            dstlocT1=d1.astype(BF), dstlocT2=d2.astype(BF),
            sqrtdeg=sq.reshape(NB, P).reshape(1, NB * P).astype(BF),
            dinv_own=dv.reshape(NB, P).T.copy(),
            batchloc=bl.reshape(NB, P).T.astype(BF),
        ))

    cnt = np.bincount(batch, minlength=G).astype(np.float32)
    invc = (1.0 / np.maximum(cnt, 1.0)).astype(np.float32)
    shared = dict(dconst=dconst.astype(BF))
    return plan, per_core, shared, invc


def _build(plan):
    nch1, nchA, nchB = plan["nch1"], plan["nchA"], plan["nchB"]
    n1_set, n2_set = plan["n1_set"], plan["n2_set"]
    nA_set, nB_set = plan["nA_set"], plan["nB_set"]
    NM1, NM2, NMX = plan["NM1"], plan["NM2"], plan["NMX"]
    NCH1 = plan["NCH1"]
    icolsA, icolsB = plan["icolsA"], plan["icolsB"]
    sched1, sched2 = plan["sched1"], plan["sched2"]

    nc = bacc.Bacc("TRN2", target_bir_lowering=False, debug=False,
                   num_devices=C, num_swdge_queues=4)

    xs_in = nc.dram_tensor("xs", [P, NCH1 * F], BF16, kind="ExternalInput")
    idxA_in = nc.dram_tensor("idxA", [P, max(icolsA, 8)], I16, kind="ExternalInput")
    idxB_in = nc.dram_tensor("idxB", [P, max(icolsB, 8)], I16, kind="ExternalInput")
    d1_in = nc.dram_tensor("dstlocT1", [NM1, NSET * P], BF16, kind="ExternalInput")
    d2_in = nc.dram_tensor("dstlocT2", [NM2, NSET * P], BF16, kind="ExternalInput")
    dc_in = nc.dram_tensor("dconst", [NMX, (NMX - 1) * P], BF16, kind="ExternalInput")
    sq_in = nc.dram_tensor("sqrtdeg", [1, NB * P], BF16, kind="ExternalInput")
    dv_in = nc.dram_tensor("dinv_own", [P, NB], F32, kind="ExternalInput")
    bl_in = nc.dram_tensor("batchloc", [P, NB], BF16, kind="ExternalInput")
    iota_in = nc.dram_tensor("iota", [P, P], BF16, kind="ExternalInput")
    W1_in = nc.dram_tensor("W1", [F, F], BF16, kind="ExternalInput")
    b1_in = nc.dram_tensor("b1r", [1, F], BF16, kind="ExternalInput")
    W2_in = nc.dram_tensor("W2", [F, F], BF16, kind="ExternalInput")
    b2_in = nc.dram_tensor("b2r", [1, F], BF16, kind="ExternalInput")
    Wfc_in = nc.dram_tensor("Wfc", [F, OUT], F32, kind="ExternalInput")
    bfc_in = nc.dram_tensor("bfcb", [P, OUT], F32, kind="ExternalInput")
    invc_in = nc.dram_tensor("invc", [F, G], F32, kind="ExternalInput")
    out = nc.dram_tensor("out", [G, OUT], F32, kind="ExternalOutput")

    gq = [0]

    with tile.TileContext(nc) as tc:
        with (
            tc.tile_pool(name="const", bufs=1) as cp,
            tc.tile_pool(name="stream", bufs=2) as stp,
            tc.tile_pool(name="gpool", bufs=2) as gp,
            tc.tile_pool(name="spool", bufs=2) as sp,
            tc.tile_pool(name="epool", bufs=4) as ep,
            tc.tile_pool(name="psA", bufs=2, space="PSUM") as psA,
            tc.tile_pool(name="psD", bufs=2, space="PSUM") as psD,
            tc.tile_pool(name="psH", bufs=2, space="PSUM") as psH,
            tc.tile_pool(name="psP", bufs=1, space="PSUM") as psP,
            tc.tile_pool(name="dram", bufs=1, space="DRAM") as dram,
        ):
            # ---- constants ----
            iota_sb = cp.tile([P, P], BF16, tag="iota")
            nc.sync.dma_start(iota_sb[:], iota_in[:])
            idxA_sb = cp.tile([P, max(icolsA, 8)], I16, tag="idxA")
            nc.scalar.dma_start(idxA_sb[:], idxA_in[:])
            idxB_sb = cp.tile([P, max(icolsB, 8)], I16, tag="idxB")
            nc.scalar.dma_start(idxB_sb[:], idxB_in[:])
            d1_sb = cp.tile([NM1, NSET * P], BF16, tag="d1")
            nc.scalar.dma_start(d1_sb[:], d1_in[:])
            d2_sb = cp.tile([NM2, NSET * P], BF16, tag="d2")
            nc.scalar.dma_start(d2_sb[:], d2_in[:])
            dc_sb = cp.tile([NMX, (NMX - 1) * P], BF16, tag="dc")
            nc.scalar.dma_start(dc_sb[:], dc_in[:])
            sq_sb = cp.tile([1, NB * P], BF16, tag="sq")
            nc.sync.dma_start(sq_sb[:], sq_in[:])
            dv_sb = cp.tile([P, NB], F32, tag="dv")
            nc.sync.dma_start(dv_sb[:], dv_in[:])
            bl_sb = cp.tile([P, NB], BF16, tag="bl")
            nc.sync.dma_start(bl_sb[:], bl_in[:])
            W1_sb = cp.tile([F, F], BF16, tag="W1")
            nc.sync.dma_start(W1_sb[:], W1_in[:])
            b1_sb = cp.tile([1, F], BF16, tag="b1")
            nc.sync.dma_start(b1_sb[:], b1_in[:])
            W2_sb = cp.tile([F, F], BF16, tag="W2")
            nc.sync.dma_start(W2_sb[:], W2_in[:])
            b2_sb = cp.tile([1, F], BF16, tag="b2")
            nc.sync.dma_start(b2_sb[:], b2_in[:])
            Wfc_sb = cp.tile([F, OUT], F32, tag="Wfc")
            nc.sync.dma_start(Wfc_sb[:], Wfc_in[:])
            bfc_sb = cp.tile([P, OUT], F32, tag="bfc")
            nc.sync.dma_start(bfc_sb[:], bfc_in[:])
            invc_sb = cp.tile([F, G], F32, tag="invc")
            nc.sync.dma_start(invc_sb[:], invc_in[:])

            own_sb = cp.tile([P, NB, F], BF16, tag="own")

            shardA = dram.tile([SA_ROWS, F], BF16)
            shardB = dram.tile([SB_ROWS, F], BF16)
            fullA = dram.tile([C * SA_ROWS, F], BF16, addr_space="Shared")
            fullB = dram.tile([C * SB_ROWS, F], BF16, addr_space="Shared")
            h1pad = dram.tile([N, 2 * F], BF16)
            pool_in = dram.tile([F, G], F32)
            pool_out = dram.tile([F, G], F32, addr_space="Shared")

            pool_ps = psP.tile([F, G], F32, tag="pool")

            # pooling one-hots (startup; DVE tensor_tensor is fine here)
            Sp_all = cp.tile([P, NB, G], BF16, tag="Sp_all")
            blm = bl_sb[:, :]
            nc.vector.tensor_tensor(
                out=Sp_all[:],
                in0=_bcast_ap(iota_sb[:], [iota_sb[:].ap[0], [0, NB], [1, G]]),
                in1=_bcast_ap(blm, [blm.ap[0], [blm.ap[1][0], NB], [0, G]]),
                op=mybir.AluOpType.is_equal,
            )

            def gen_S(s, n_set, d_sb, tag):
                """S[p, k*P+d] = (dstloc[p,k] == d) via PE D-matmul + DVE."""
                S_t = sp.tile([P, n_set * P], BF16, tag=tag)
                ncols = n_set * P
                for p0 in range(0, ncols, 512):
                    pc = min(512, ncols - p0)
                    Dp = psD.tile([P, pc], F32, tag="D")
                    nc.tensor.matmul(
                        Dp[:],
                        lhsT=d_sb[0:n_set + 1, s * P:(s + 1) * P],
                        rhs=dc_sb[0:n_set + 1, p0:p0 + pc],
                        start=True, stop=True,
                    )
                    nc.vector.tensor_scalar(
                        out=S_t[:, p0:p0 + pc], in0=Dp[:],
                        scalar1=0.0, scalar2=None,
                        op0=mybir.AluOpType.is_equal,
                    )
                return S_t

            def epilogue(b, aggT, W_sb, brow_sb, layer):
                agg_sb = ep.tile([F, P], BF16, tag="agg_sb")
                nc.vector.tensor_copy(agg_sb[:], aggT[:])
                h_ps = psH.tile([P, F], F32, tag="h")
                nc.tensor.matmul(h_ps[:], lhsT=agg_sb[:], rhs=W_sb[:],
                                 start=True, stop=False)
                nc.tensor.matmul(h_ps[:], lhsT=sq_sb[0:1, b * P:(b + 1) * P],
                                 rhs=brow_sb[:], start=False, stop=True)
                if layer == 1:
                    h1t = ep.tile([P, F], BF16, tag="h1t")
                    nc.scalar.activation(h1t[:], h_ps[:],
                                         mybir.ActivationFunctionType.Tanh,
                                         scale=dv_sb[:, b:b+1])
                    # h~1 = dinv * h1 -> persists in own_sb, written to shard
                    nc.scalar.activation(own_sb[:, b, :], h1t[:],
                                         mybir.ActivationFunctionType.Copy,
                                         scale=dv_sb[:, b:b+1])
                    if b < ABLOCKS:
                        r0 = b * P
                        nc.sync.dma_start(shardA[r0:r0 + P, :], own_sb[:, b, :])
                    else:
                        r0 = (b - ABLOCKS) * P
                        rows = min(P, SB_ROWS - r0)
                        nc.sync.dma_start(shardB[r0:r0 + rows, :],
                                          own_sb[:rows, b, :])
                else:
                    h2t = ep.tile([P, F], BF16, tag="h2t")
                    nc.scalar.activation(h2t[:], h_ps[:],
                                         mybir.ActivationFunctionType.Tanh,
                                         scale=dv_sb[:, b:b+1])
                    nc.tensor.matmul(pool_ps[:], lhsT=h2t[:],
                                     rhs=Sp_all[:, b, :],
                                     start=(b == 0), stop=(b == NB - 1),
                                     skip_group_check=True)

            # =================== Layer 1 (streamed) ===================
            ch_off = 0
            for s in range(NSET):
                n1 = int(n1_set[s])
                st = stp.tile([P, n1 * F], BF16, tag="st")
                nc.sync.dma_start(st[:], xs_in[:, ch_off * F:(ch_off + n1) * F])
                S1 = gen_S(s, n1, d1_sb, "S1")
                mms = sched1[s]
                # group by block
                blocks = _set_blocks(s)
                for b in blocks:
                    kis = [k for k, (bb, kind, col) in enumerate(mms) if bb == b]
                    aggT = psA.tile([F, P], F32, tag="aggT")
                    for j, k in enumerate(kis):
                        _, _, col = mms[k]
                        nc.tensor.matmul(
                            aggT[:],
                            lhsT=st[:, col * F:(col + 1) * F],
                            rhs=S1[:, k * P:(k + 1) * P],
                            start=(j == 0), stop=(j == len(kis) - 1),
                        )
                    epilogue(b, aggT, W1_sb, b1_sb, 1)
                    if b == ABLOCKS - 1:
                        nc.gpsimd.collective_compute(
                            "AllGather", mybir.AluOpType.bypass,
                            ins=[shardA.opt()], outs=[fullA.opt()],
                            replica_groups=[list(range(C))],
                        )
                        for cc in range(C):
                            nc.scalar.dma_start(
                                h1pad[cc * NSH:cc * NSH + SA_ROWS, 0:F],
                                fullA[cc * SA_ROWS:(cc + 1) * SA_ROWS, :])
                ch_off += n1

            nc.gpsimd.collective_compute(
                "AllGather", mybir.AluOpType.bypass,
                ins=[shardB.opt()], outs=[fullB.opt()],
                replica_groups=[list(range(C))],
            )
            for cc in range(C):
                nc.scalar.dma_start(
                    h1pad[cc * NSH + SA_ROWS:(cc + 1) * NSH, 0:F],
                    fullB[cc * SB_ROWS:(cc + 1) * SB_ROWS, :])

            # =================== Layer 2 (gathered) ===================
            acol = bcol = 0
            for s in range(NSET):
                n2 = int(n2_set[s])
                nAs, nBs = int(nA_set[s]), int(nB_set[s])
                gtA = gp.tile([P, nAs, 2 * F], BF16, tag="gtA")
                q = gq[0] % 4
                gq[0] += 1
                nc.gpsimd.dma_gather(
                    gtA[:], h1pad[0:ASPLIT, :],
                    idxA_sb[:, acol:acol + nAs * (P // 16)],
                    nAs * P, nAs * P, 2 * F,
                    single_packet=False, queue_num=q,
                )
                gtB = gp.tile([P, nBs, 2 * F], BF16, tag="gtB")
                q = gq[0] % 4
                gq[0] += 1
                nc.gpsimd.dma_gather(
                    gtB[:], h1pad[ASPLIT:N, :],
                    idxB_sb[:, bcol:bcol + nBs * (P // 16)],
                    nBs * P, nBs * P, 2 * F,
                    single_packet=False, queue_num=q,
                )
                S2 = gen_S(s, n2, d2_sb, "S2")
                mms = sched2[s]
                blocks = _set_blocks(s)
                for b in blocks:
                    kis = [k for k, (bb, kind, col) in enumerate(mms) if bb == b]
                    aggT = psA.tile([F, P], F32, tag="aggT")
                    for j, k in enumerate(kis):
                        _, kind, col = mms[k]
                        if kind == 0:
                            lhsT = own_sb[:, b, :]
                        elif kind == 1:
                            lhsT = gtA[:, col, 0:F]
                        else:
                            lhsT = gtB[:, col, 0:F]
                        nc.tensor.matmul(
                            aggT[:], lhsT=lhsT,
                            rhs=S2[:, k * P:(k + 1) * P],
                            start=(j == 0), stop=(j == len(kis) - 1),
                        )
                    epilogue(b, aggT, W2_sb, b2_sb, 2)
                acol += nAs * (P // 16)
                bcol += nBs * (P // 16)

            # ---- pooled tail ----
            poolT = ep.tile([F, G], F32, tag="poolT")
            nc.vector.tensor_copy(poolT[:], pool_ps[:])
            nc.sync.dma_start(pool_in[:], poolT[:])
            nc.gpsimd.collective_compute(
                "AllReduce", mybir.AluOpType.add,
                ins=[pool_in.opt()], outs=[pool_out.opt()],
                replica_groups=[list(range(C))],
            )
            poolR = ep.tile([F, G], F32, tag="poolR")
            nc.sync.dma_start(poolR[:], pool_out[:])
            nc.vector.tensor_mul(poolR[:], poolR[:], invc_sb[:])
            fc_ps = psP.tile([G, OUT], F32, tag="fc")
            nc.tensor.matmul(fc_ps[:], lhsT=poolR[:], rhs=Wfc_sb[:],
                             start=True, stop=True)
            out_sb = ep.tile([G, OUT], F32, tag="out_sb")
            nc.vector.tensor_add(out_sb[:], fc_ps[:], bfc_sb[:])
            nc.sync.dma_start(out[:], out_sb[:])

    nc.compile()
    return nc


def _in_maps(plan, per_core, shared, invc, W1, b1, W2, b2, Wfc, bfc):
    iota = np.tile(np.arange(P, dtype=np.float32), (P, 1)).astype(BF)
    icolsA, icolsB = plan["icolsA"], plan["icolsB"]
    com = dict(
        iota=iota,
        dconst=shared["dconst"],
        W1=np.asarray(W1, np.float32).astype(BF),
        b1r=np.asarray(b1, np.float32).reshape(1, F).astype(BF),
        W2=np.asarray(W2, np.float32).astype(BF),
        b2r=np.asarray(b2, np.float32).reshape(1, F).astype(BF),
        Wfc=np.ascontiguousarray(np.asarray(Wfc, np.float32)),
        bfcb=np.tile(np.asarray(bfc, np.float32), (P, 1)),
        invc=np.tile(invc, (F, 1)),
    )
    maps = []
    for c in range(C):
        m = dict(com)
        pc = dict(per_core[c])
        if icolsA == 0:
            pc["idxA"] = np.zeros((P, 8), np.int16)
        if icolsB == 0:
            pc["idxB"] = np.zeros((P, 8), np.int16)
        m.update(pc)
        maps.append({k: np.ascontiguousarray(v) for k, v in m.items()})
    return maps


_RUN_KWARGS = {}


def kernel(x, src, dst, batch, W1, b1, W2, b2, Wfc, bfc):
    plan, per_core, shared, invc = _preprocess(x, src, dst, batch)
    nc = _build(plan)
    maps = _in_maps(plan, per_core, shared, invc, W1, b1, W2, b2, Wfc, bfc)
    res = bass_utils.run_bass_kernel_spmd(
        nc, maps, core_ids=list(range(C)), **_RUN_KWARGS
    )
    kernel.last_results = res
    return np.asarray(res.results[0]["out"], np.float32)


# revision 5
# speedup vs baseline: 1.3796x; 1.2514x over previous
"""Trainium2 Bass kernel for a 2-layer GCN + global mean pool + FC.

v3 strategy (8 NeuronCores, SPMD single NEFF):
  - Nodes (and in-edges) partitioned by dst across 8 cores.
  - A single unified chunk plan for both layers: per dst block, A-half and
    B-half gather chunks (split at global row 32768 for int16 indices) plus
    one self chunk.  The norm-hot routing masks S (norm_e at the edge's
    dst column, 1/deg on the self diagonal) depend only on graph structure,
    are identical for both layers, and are built on the host and streamed
    from HBM (no on-device mask generation at all).
  - Layer 1 messages are host-expanded into a contiguous per-edge stream of
    raw bf16 x rows in the same chunk order -- no gathers, no Q7 work.
  - Layer 2 gathers raw bf16 h1 rows with dma_gather from a 256B-padded
    table (h1pad[N,128]); indices sorted ascending per call.
  - agg is accumulated transposed: aggT[64f,128d] += tile^T @ S_chunk, so
    the epilogue is a direct matmul with W (bias via a rank-1 ones x b
    matmul) followed by one ACT tanh -- no transposes, no DVE in the
    steady state (DVE 2-port ops would lock GpSimd out of SBUF and stall
    descriptor generation).
  - AllGather of h1 split in two halves (first issued mid-layer-1), then
    HWDGE row-strided expand into h1pad.
"""

import numpy as np
import ml_dtypes

from concourse import bacc, bass, mybir, bass_utils
import concourse.tile as tile

N = 50000
E = 800000
F = 64
G = 128
OUT = 8
P = 128
C = 8
NSH = N // C          # 6250 nodes per core
NB = (NSH + P - 1) // P   # 49 dst blocks per core
SBLK = 4
NSET = (NB + SBLK - 1) // SBLK  # 13 sets
ASPLIT = 32768        # gather A/B split (int16 index limit)
ABLOCKS = 25          # shard-A blocks (AG split)
SA_ROWS = ABLOCKS * P         # 3200
SB_ROWS = NSH - SA_ROWS       # 3050
F32 = mybir.dt.float32
BF16 = mybir.dt.bfloat16
I16 = mybir.dt.int16
BF = ml_dtypes.bfloat16


def _set_blocks(s):
    return list(range(s * SBLK, min((s + 1) * SBLK, NB)))


def _preprocess(x, src, dst, batch):
    """Host-side planning: index work + layout transforms of the inputs."""
    src = np.asarray(src).astype(np.int64)
    dst = np.asarray(dst).astype(np.int64)
    batch = np.asarray(batch).astype(np.int64)
    xb = np.asarray(x, np.float32).astype(BF)

    deg = np.bincount(dst, minlength=N).astype(np.float64) + 1.0
    dinv = 1.0 / np.sqrt(deg)

    # per-core, per-(block, half) edge lists sorted by src
    core_e = []     # [c][b] -> (esA, dlA, esB, dlB)
    for c in range(C):
        lo = c * NSH
        m = (dst >= lo) & (dst < lo + NSH)
        es, ed = src[m], dst[m] - lo
        blk = ed >> 7
        dl = ed & 127
        order = np.lexsort((es, blk))
        es, dl, blk = es[order], dl[order], blk[order]
        bounds = np.searchsorted(blk, np.arange(NB + 1))
        per_b = []
        for b in range(NB):
            g0, g1 = bounds[b], bounds[b + 1]
            e, d = es[g0:g1], dl[g0:g1]
            ma = e < ASPLIT
            per_b.append((e[ma], d[ma], e[~ma], d[~ma]))
        core_e.append(per_b)

    cntA = np.zeros((C, NB), np.int64)
    cntB = np.zeros((C, NB), np.int64)
    for c in range(C):
        for b in range(NB):
            eA, _, eB, _ = core_e[c][b]
            cntA[c, b] = len(eA)
            cntB[c, b] = len(eB)
    nchA = np.maximum(np.ceil(cntA.max(axis=0) / P).astype(np.int64), 1)
    nchB = np.maximum(np.ceil(cntB.max(axis=0) / P).astype(np.int64), 1)

    n_set = np.array([sum(1 + nchA[b] + nchB[b] for b in _set_blocks(s))
                      for s in range(NSET)])
    NCHT = int(n_set.sum())
    nA_set = np.array([sum(nchA[b] for b in _set_blocks(s)) for s in range(NSET)])
    nB_set = np.array([sum(nchB[b] for b in _set_blocks(s)) for s in range(NSET)])
    icolsA = int(nA_set.sum()) * (P // 16)
    icolsB = int(nB_set.sum()) * (P // 16)

    # mm schedule per set: (block, kind, tile_col); kind 0=self 1=A 2=B
    sched = []
    for s in range(NSET):
        lst = []
        ao = bo = 0
        for b in _set_blocks(s):
            lst.append((b, 0, 0))
            for i in range(int(nchA[b])):
                lst.append((b, 1, ao)); ao += 1
            for i in range(int(nchB[b])):
                lst.append((b, 2, bo)); bo += 1
        sched.append(lst)

    plan = dict(nchA=nchA, nchB=nchB, n_set=n_set, nA_set=nA_set,
                nB_set=nB_set, NCHT=NCHT, icolsA=icolsA, icolsB=icolsB,
                sched=sched)

    per_core = []
    for c in range(C):
        xs = np.zeros((P, NCHT, F), BF)
        Sm = np.zeros((P, NCHT, P), BF)
        idxA_parts, idxB_parts = [], []
        ch = 0
        for s in range(NSET):
            for b in _set_blocks(s):
                eA, dA, eB, dB = core_e[c][b]
                # self chunk
                nr = min(P, NSH - b * P)
                own = c * NSH + b * P + np.arange(nr)
                xs[:nr, ch, :] = xb[own]
                Sm[np.arange(nr), ch, np.arange(nr)] = (1.0 / deg[own]).astype(BF)
                ch += 1
                for i in range(int(nchA[b])):
                    rows = eA[i * P:(i + 1) * P]
                    dls = dA[i * P:(i + 1) * P]
                    nr = len(rows)
                    gi = np.zeros(P, np.int64)
                    gi[:nr] = rows
                    idxA_parts.append(gi)
                    if nr:
                        xs[:nr, ch, :] = xb[rows]
                        nrm = (dinv[rows] * dinv[c * NSH + b * P + dls]).astype(BF)
                        Sm[np.arange(nr), ch, dls] = nrm
                    ch += 1
                for i in range(int(nchB[b])):
                    rows = eB[i * P:(i + 1) * P]
                    dls = dB[i * P:(i + 1) * P]
                    nr = len(rows)
                    gi = np.zeros(P, np.int64)
                    gi[:nr] = rows - ASPLIT
                    idxB_parts.append(gi)
                    if nr:
                        xs[:nr, ch, :] = xb[rows]
                        nrm = (dinv[rows] * dinv[c * NSH + b * P + dls]).astype(BF)
                        Sm[np.arange(nr), ch, dls] = nrm
                    ch += 1
        assert ch == NCHT

        def mk_idx(parts):
            if not parts:
                return np.zeros((P, 8), np.int16)
            stk = np.concatenate(parts).astype(np.int16)
            return np.tile(stk.reshape(-1, 16).T, (8, 1))

        # pooling one-hots [P, NB, G]
        Sp = np.zeros((P, NB, G), BF)
        own = np.arange(NSH)
        Sp[own & 127, own >> 7, batch[c * NSH + own]] = 1.0
        per_core.append(dict(
            xs=np.ascontiguousarray(xs.reshape(P, NCHT * F)),
            Sm=np.ascontiguousarray(Sm.reshape(P, NCHT * P)),
            idxA=mk_idx(idxA_parts), idxB=mk_idx(idxB_parts),
            Sp=np.ascontiguousarray(Sp),
        ))

    cnt = np.bincount(batch, minlength=G).astype(np.float32)
    invc = (1.0 / np.maximum(cnt, 1.0)).astype(np.float32)
    return plan, per_core, invc


def _build(plan):
    nchA, nchB = plan["nchA"], plan["nchB"]
    n_set, nA_set, nB_set = plan["n_set"], plan["nA_set"], plan["nB_set"]
    NCHT = plan["NCHT"]
    icolsA, icolsB = plan["icolsA"], plan["icolsB"]
    sched = plan["sched"]

    nc = bacc.Bacc("TRN2", target_bir_lowering=False, debug=False,
                   num_devices=C, num_swdge_queues=4)

    xs_in = nc.dram_tensor("xs", [P, NCHT * F], BF16, kind="ExternalInput")
    Sm_in = nc.dram_tensor("Sm", [P, NCHT * P], BF16, kind="ExternalInput")
    idxA_in = nc.dram_tensor("idxA", [P, max(icolsA, 8)], I16, kind="ExternalInput")
    idxB_in = nc.dram_tensor("idxB", [P, max(icolsB, 8)], I16, kind="ExternalInput")
    Sp_in = nc.dram_tensor("Sp", [P, NB * G], BF16, kind="ExternalInput")
    W1_in = nc.dram_tensor("W1", [F, F], BF16, kind="ExternalInput")
    b1_in = nc.dram_tensor("b1r", [1, F], BF16, kind="ExternalInput")
    W2_in = nc.dram_tensor("W2", [F, F], BF16, kind="ExternalInput")
    b2_in = nc.dram_tensor("b2r", [1, F], BF16, kind="ExternalInput")
    Wfc_in = nc.dram_tensor("Wfc", [F, OUT], F32, kind="ExternalInput")
    bfc_in = nc.dram_tensor("bfcb", [P, OUT], F32, kind="ExternalInput")
    invc_in = nc.dram_tensor("invc", [F, G], F32, kind="ExternalInput")
    out = nc.dram_tensor("out", [G, OUT], F32, kind="ExternalOutput")

    gq = [0]

    with tile.TileContext(nc) as tc:
        with (
            tc.tile_pool(name="const", bufs=1) as cp,
            tc.tile_pool(name="stream", bufs=2) as stp,
            tc.tile_pool(name="smask", bufs=2) as smp,
            tc.tile_pool(name="gpool", bufs=2) as gp,
            tc.tile_pool(name="epool", bufs=4) as ep,
            tc.tile_pool(name="psA", bufs=3, space="PSUM") as psA,
            tc.tile_pool(name="psH", bufs=2, space="PSUM") as psH,
            tc.tile_pool(name="psP", bufs=1, space="PSUM") as psP,
            tc.tile_pool(name="dram", bufs=1, space="DRAM") as dram,
        ):
            # ---- constants ----
            idxA_sb = cp.tile([P, max(icolsA, 8)], I16, tag="idxA")
            nc.scalar.dma_start(idxA_sb[:], idxA_in[:])
            idxB_sb = cp.tile([P, max(icolsB, 8)], I16, tag="idxB")
            nc.scalar.dma_start(idxB_sb[:], idxB_in[:])
            Sp_sb = cp.tile([P, NB, G], BF16, tag="Sp")
            nc.scalar.dma_start(Sp_sb[:], Sp_in[:])
            W1_sb = cp.tile([F, F], BF16, tag="W1")
            nc.sync.dma_start(W1_sb[:], W1_in[:])
            b1_sb = cp.tile([1, F], BF16, tag="b1")
            nc.sync.dma_start(b1_sb[:], b1_in[:])
            W2_sb = cp.tile([F, F], BF16, tag="W2")
            nc.sync.dma_start(W2_sb[:], W2_in[:])
            b2_sb = cp.tile([1, F], BF16, tag="b2")
            nc.sync.dma_start(b2_sb[:], b2_in[:])
            Wfc_sb = cp.tile([F, OUT], F32, tag="Wfc")
            nc.sync.dma_start(Wfc_sb[:], Wfc_in[:])
            bfc_sb = cp.tile([P, OUT], F32, tag="bfc")
            nc.sync.dma_start(bfc_sb[:], bfc_in[:])
            invc_sb = cp.tile([F, G], F32, tag="invc")
            nc.sync.dma_start(invc_sb[:], invc_in[:])
            ones_sb = cp.tile([1, P], BF16, tag="ones")
            nc.vector.memset(ones_sb[:], 1.0)

            own_sb = cp.tile([P, NB, F], BF16, tag="own")

            shardA = dram.tile([SA_ROWS, F], BF16)
            shardB = dram.tile([SB_ROWS, F], BF16)
            fullA = dram.tile([C * SA_ROWS, F], BF16, addr_space="Shared")
            fullB = dram.tile([C * SB_ROWS, F], BF16, addr_space="Shared")
            h1pad = dram.tile([N, 2 * F], BF16)
            pool_in = dram.tile([F, G], F32)
            pool_out = dram.tile([F, G], F32, addr_space="Shared")

            pool_ps = psP.tile([F, G], F32, tag="pool")

            def epilogue(b, aggT, W_sb, brow_sb, layer):
                agg_sb = ep.tile([F, P], BF16, tag="agg_sb")
                nc.scalar.copy(agg_sb[:], aggT[:])
                h_ps = psH.tile([P, F], F32, tag="h")
                nc.tensor.matmul(h_ps[:], lhsT=agg_sb[:], rhs=W_sb[:],
                                 start=True, stop=False)
                nc.tensor.matmul(h_ps[:], lhsT=ones_sb[:], rhs=brow_sb[:],
                                 start=False, stop=True)
                if layer == 1:
                    nc.scalar.activation(own_sb[:, b, :], h_ps[:],
                                         mybir.ActivationFunctionType.Tanh)
                    if b < ABLOCKS:
                        r0 = b * P
                        nc.sync.dma_start(shardA[r0:r0 + P, :], own_sb[:, b, :])
                    else:
                        r0 = (b - ABLOCKS) * P
                        rows = min(P, SB_ROWS - r0)
                        nc.sync.dma_start(shardB[r0:r0 + rows, :],
                                          own_sb[:rows, b, :])
                else:
                    h2t = ep.tile([P, F], BF16, tag="h2t")
                    nc.scalar.activation(h2t[:], h_ps[:],
                                         mybir.ActivationFunctionType.Tanh)
                    nc.tensor.matmul(pool_ps[:], lhsT=h2t[:],
                                     rhs=Sp_sb[:, b, :],
                                     start=(b == 0), stop=(b == NB - 1),
                                     skip_group_check=True)

            # =================== Layer 1 (streamed) ===================
            ch_off = 0
            for s in range(NSET):
                ns = int(n_set[s])
                st = stp.tile([P, ns * F], BF16, tag="st")
                nc.sync.dma_start(st[:], xs_in[:, ch_off * F:(ch_off + ns) * F])
                S_t = smp.tile([P, ns * P], BF16, tag="Sm")
                nc.sync.dma_start(S_t[:], Sm_in[:, ch_off * P:(ch_off + ns) * P])
                mms = sched[s]
                for b in _set_blocks(s):
                    kis = [k for k, (bb, kind, col) in enumerate(mms) if bb == b]
                    aggT = psA.tile([F, P], F32, tag="aggT")
                    for j, k in enumerate(kis):
                        nc.tensor.matmul(
                            aggT[:],
                            lhsT=st[:, k * F:(k + 1) * F],
                            rhs=S_t[:, k * P:(k + 1) * P],
                            start=(j == 0), stop=(j == len(kis) - 1),
                        )
                    epilogue(b, aggT, W1_sb, b1_sb, 1)
                    if b == ABLOCKS - 1:
                        nc.gpsimd.collective_compute(
                            "AllGather", mybir.AluOpType.bypass,
                            ins=[shardA.opt()], outs=[fullA.opt()],
                            replica_groups=[list(range(C))],
                        )
                        for cc in range(C):
                            nc.scalar.dma_start(
                                h1pad[cc * NSH:cc * NSH + SA_ROWS, 0:F],
                                fullA[cc * SA_ROWS:(cc + 1) * SA_ROWS, :])
                ch_off += ns

            nc.gpsimd.collective_compute(
                "AllGather", mybir.AluOpType.bypass,
                ins=[shardB.opt()], outs=[fullB.opt()],
                replica_groups=[list(range(C))],
            )
            for cc in range(C):
                nc.scalar.dma_start(
                    h1pad[cc * NSH + SA_ROWS:(cc + 1) * NSH, 0:F],
                    fullB[cc * SB_ROWS:(cc + 1) * SB_ROWS, :])

            # =================== Layer 2 (gathered) ===================
            acol = bcol = 0
            ch_off = 0
            for s in range(NSET):
                ns = int(n_set[s])
                nAs, nBs = int(nA_set[s]), int(nB_set[s])
                gtA = gp.tile([P, nAs, 2 * F], BF16, tag="gtA")
                q = gq[0] % 4
                gq[0] += 1
                nc.gpsimd.dma_gather(
                    gtA[:], h1pad[0:ASPLIT, :],
                    idxA_sb[:, acol:acol + nAs * (P // 16)],
                    nAs * P, nAs * P, 2 * F,
                    single_packet=False, queue_num=q,
                )
                gtB = gp.tile([P, nBs, 2 * F], BF16, tag="gtB")
                q = gq[0] % 4
                gq[0] += 1
                nc.gpsimd.dma_gather(
                    gtB[:], h1pad[ASPLIT:N, :],
                    idxB_sb[:, bcol:bcol + nBs * (P // 16)],
                    nBs * P, nBs * P, 2 * F,
                    single_packet=False, queue_num=q,
                )
                S_t = smp.tile([P, ns * P], BF16, tag="Sm")
                nc.sync.dma_start(S_t[:], Sm_in[:, ch_off * P:(ch_off + ns) * P])
                mms = sched[s]
                for b in _set_blocks(s):
                    kis = [k for k, (bb, kind, col) in enumerate(mms) if bb == b]
                    aggT = psA.tile([F, P], F32, tag="aggT")
                    for j, k in enumerate(kis):
                        _, kind, col = mms[k]
                        if kind == 0:
                            lhsT = own_sb[:, b, :]
                        elif kind == 1:
                            lhsT = gtA[:, col, 0:F]
                        else:
                            lhsT = gtB[:, col, 0:F]
                        nc.tensor.matmul(
                            aggT[:], lhsT=lhsT,
                            rhs=S_t[:, k * P:(k + 1) * P],
                            start=(j == 0), stop=(j == len(kis) - 1),
                        )
                    epilogue(b, aggT, W2_sb, b2_sb, 2)
                acol += nAs * (P // 16)
                bcol += nBs * (P // 16)
                ch_off += ns

            # ---- pooled tail ----
            poolT = ep.tile([F, G], F32, tag="poolT")
            nc.vector.tensor_copy(poolT[:], pool_ps[:])
            nc.sync.dma_start(pool_in[:], poolT[:])
            nc.gpsimd.collective_compute(
                "AllReduce", mybir.AluOpType.add,
                ins=[pool_in.opt()], outs=[pool_out.opt()],
                replica_groups=[list(range(C))],
            )
            poolR = ep.tile([F, G], F32, tag="poolR")
            nc.sync.dma_start(poolR[:], pool_out[:])
            nc.vector.tensor_mul(poolR[:], poolR[:], invc_sb[:])
            fc_ps = psP.tile([G, OUT], F32, tag="fc")
            nc.tensor.matmul(fc_ps[:], lhsT=poolR[:], rhs=Wfc_sb[:],
                             start=True, stop=True)
            out_sb = ep.tile([G, OUT], F32, tag="out_sb")
            nc.vector.tensor_add(out_sb[:], fc_ps[:], bfc_sb[:])
            nc.sync.dma_start(out[:], out_sb[:])

    nc.compile()
    return nc


def _in_maps(plan, per_core, invc, W1, b1, W2, b2, Wfc, bfc):
    com = dict(
        W1=np.asarray(W1, np.float32).astype(BF),
        b1r=np.asarray(b1, np.float32).reshape(1, F).astype(BF),
        W2=np.asarray(W2, np.float32).astype(BF),
        b2r=np.asarray(b2, np.float32).reshape(1, F).astype(BF),
        Wfc=np.ascontiguousarray(np.asarray(Wfc, np.float32)),
        bfcb=np.tile(np.asarray(bfc, np.float32), (P, 1)),
        invc=np.tile(invc, (F, 1)),
    )
    maps = []
    for c in range(C):
        m = dict(com)
        m.update(per_core[c])
        m["Sp"] = m["Sp"].reshape(P, NB * G)
        maps.append({k: np.ascontiguousarray(v) for k, v in m.items()})
    return maps


_RUN_KWARGS = {}


def kernel(x, src, dst, batch, W1, b1, W2, b2, Wfc, bfc):
    plan, per_core, invc = _preprocess(x, src, dst, batch)
    nc = _build(plan)
    maps = _in_maps(plan, per_core, invc, W1, b1, W2, b2, Wfc, bfc)
    res = bass_utils.run_bass_kernel_spmd(
        nc, maps, core_ids=list(range(C)), **_RUN_KWARGS
    )
    kernel.last_results = res
    return np.asarray(res.results[0]["out"], np.float32)


# revision 6
# speedup vs baseline: 1.4046x; 1.0181x over previous
"""Trainium2 Bass kernel for a 2-layer GCN + global mean pool + FC.

v3 strategy (8 NeuronCores, SPMD single NEFF):
  - Nodes (and in-edges) partitioned by dst across 8 cores.
  - A single unified chunk plan for both layers: per dst block, A-half and
    B-half gather chunks (split at global row 32768 for int16 indices) plus
    one self chunk.  The norm-hot routing masks S (norm_e at the edge's
    dst column, 1/deg on the self diagonal) depend only on graph structure,
    are identical for both layers, and are built on the host and streamed
    from HBM (no on-device mask generation at all).
  - Layer 1 messages are host-expanded into a contiguous per-edge stream of
    raw bf16 x rows in the same chunk order -- no gathers, no Q7 work.
  - Layer 2 gathers raw bf16 h1 rows with dma_gather from a 256B-padded
    table (h1pad[N,128]); indices sorted ascending per call.
  - agg is accumulated transposed: aggT[64f,128d] += tile^T @ S_chunk, so
    the epilogue is a direct matmul with W (bias via a rank-1 ones x b
    matmul) followed by one ACT tanh -- no transposes, no DVE in the
    steady state (DVE 2-port ops would lock GpSimd out of SBUF and stall
    descriptor generation).
  - AllGather of h1 split in two halves (first issued mid-layer-1), then
    HWDGE row-strided expand into h1pad.
"""

import numpy as np
import ml_dtypes

from concourse import bacc, bass, mybir, bass_utils
import concourse.tile as tile

N = 50000
E = 800000
F = 64
G = 128
OUT = 8
P = 128
C = 8
NSH = N // C          # 6250 nodes per core
NB = (NSH + P - 1) // P   # 49 dst blocks per core
SBLK = 4
NSET = (NB + SBLK - 1) // SBLK  # 13 sets
ASPLIT = 32768        # gather A/B split (int16 index limit)
ABLOCKS = 25          # shard-A blocks (AG split)
SA_ROWS = ABLOCKS * P         # 3200
SB_ROWS = NSH - SA_ROWS       # 3050
F32 = mybir.dt.float32
BF16 = mybir.dt.bfloat16
I16 = mybir.dt.int16
BF = ml_dtypes.bfloat16


def _pieces(n, k):
    out = []
    step = (n + k - 1) // k
    for c0 in range(0, n, step):
        out.append((c0, min(c0 + step, n)))
    return out


def _set_blocks(s):
    return list(range(s * SBLK, min((s + 1) * SBLK, NB)))


def _preprocess(x, src, dst, batch):
    """Host-side planning: index work + layout transforms of the inputs."""
    src = np.asarray(src).astype(np.int64)
    dst = np.asarray(dst).astype(np.int64)
    batch = np.asarray(batch).astype(np.int64)
    xb = np.asarray(x, np.float32).astype(BF)

    deg = np.bincount(dst, minlength=N).astype(np.float64) + 1.0
    dinv = 1.0 / np.sqrt(deg)

    # per-core, per-(block, half) edge lists sorted by src
    core_e = []     # [c][b] -> (esA, dlA, esB, dlB)
    for c in range(C):
        lo = c * NSH
        m = (dst >= lo) & (dst < lo + NSH)
        es, ed = src[m], dst[m] - lo
        blk = ed >> 7
        dl = ed & 127
        order = np.lexsort((es, blk))
        es, dl, blk = es[order], dl[order], blk[order]
        bounds = np.searchsorted(blk, np.arange(NB + 1))
        per_b = []
        for b in range(NB):
            g0, g1 = bounds[b], bounds[b + 1]
            e, d = es[g0:g1], dl[g0:g1]
            ma = e < ASPLIT
            per_b.append((e[ma], d[ma], e[~ma], d[~ma]))
        core_e.append(per_b)

    cntA = np.zeros((C, NB), np.int64)
    cntB = np.zeros((C, NB), np.int64)
    for c in range(C):
        for b in range(NB):
            eA, _, eB, _ = core_e[c][b]
            cntA[c, b] = len(eA)
            cntB[c, b] = len(eB)
    nchA = np.maximum(np.ceil(cntA.max(axis=0) / P).astype(np.int64), 1)
    nchB = np.maximum(np.ceil(cntB.max(axis=0) / P).astype(np.int64), 1)

    n_set = np.array([sum(1 + nchA[b] + nchB[b] for b in _set_blocks(s))
                      for s in range(NSET)])
    NCHT = int(n_set.sum())
    nA_set = np.array([sum(nchA[b] for b in _set_blocks(s)) for s in range(NSET)])
    nB_set = np.array([sum(nchB[b] for b in _set_blocks(s)) for s in range(NSET)])
    icolsA = int(nA_set.sum()) * (P // 16)
    icolsB = int(nB_set.sum()) * (P // 16)

    # mm schedule per set: (block, kind, tile_col); kind 0=self 1=A 2=B
    sched = []
    for s in range(NSET):
        lst = []
        ao = bo = 0
        for b in _set_blocks(s):
            lst.append((b, 0, 0))
            for i in range(int(nchA[b])):
                lst.append((b, 1, ao)); ao += 1
            for i in range(int(nchB[b])):
                lst.append((b, 2, bo)); bo += 1
        sched.append(lst)

    plan = dict(nchA=nchA, nchB=nchB, n_set=n_set, nA_set=nA_set,
                nB_set=nB_set, NCHT=NCHT, icolsA=icolsA, icolsB=icolsB,
                sched=sched)

    per_core = []
    for c in range(C):
        xs = np.zeros((P, NCHT, F), BF)
        Sm = np.zeros((P, NCHT, P), BF)
        idxA_parts, idxB_parts = [], []
        ch = 0
        for s in range(NSET):
            for b in _set_blocks(s):
                eA, dA, eB, dB = core_e[c][b]
                # self chunk
                nr = min(P, NSH - b * P)
                own = c * NSH + b * P + np.arange(nr)
                xs[:nr, ch, :] = xb[own]
                Sm[np.arange(nr), ch, np.arange(nr)] = (1.0 / deg[own]).astype(BF)
                ch += 1
                for i in range(int(nchA[b])):
                    rows = eA[i * P:(i + 1) * P]
                    dls = dA[i * P:(i + 1) * P]
                    nr = len(rows)
                    gi = np.zeros(P, np.int64)
                    gi[:nr] = rows
                    idxA_parts.append(gi)
                    if nr:
                        xs[:nr, ch, :] = xb[rows]
                        nrm = (dinv[rows] * dinv[c * NSH + b * P + dls]).astype(BF)
                        Sm[np.arange(nr), ch, dls] = nrm
                    ch += 1
                for i in range(int(nchB[b])):
                    rows = eB[i * P:(i + 1) * P]
                    dls = dB[i * P:(i + 1) * P]
                    nr = len(rows)
                    gi = np.zeros(P, np.int64)
                    gi[:nr] = rows - ASPLIT
                    idxB_parts.append(gi)
                    if nr:
                        xs[:nr, ch, :] = xb[rows]
                        nrm = (dinv[rows] * dinv[c * NSH + b * P + dls]).astype(BF)
                        Sm[np.arange(nr), ch, dls] = nrm
                    ch += 1
        assert ch == NCHT

        def mk_idx(parts):
            if not parts:
                return np.zeros((P, 8), np.int16)
            stk = np.concatenate(parts).astype(np.int16)
            return np.tile(stk.reshape(-1, 16).T, (8, 1))

        # pooling one-hots [P, NB, G]
        Sp = np.zeros((P, NB, G), BF)
        own = np.arange(NSH)
        Sp[own & 127, own >> 7, batch[c * NSH + own]] = 1.0
        per_core.append(dict(
            xs=np.ascontiguousarray(xs.reshape(P, NCHT * F)),
            Sm=np.ascontiguousarray(Sm.reshape(P, NCHT * P)),
            idxA=mk_idx(idxA_parts), idxB=mk_idx(idxB_parts),
            Sp=np.ascontiguousarray(Sp),
        ))

    cnt = np.bincount(batch, minlength=G).astype(np.float32)
    invc = (1.0 / np.maximum(cnt, 1.0)).astype(np.float32)
    return plan, per_core, invc


def _build(plan):
    nchA, nchB = plan["nchA"], plan["nchB"]
    n_set, nA_set, nB_set = plan["n_set"], plan["nA_set"], plan["nB_set"]
    NCHT = plan["NCHT"]
    icolsA, icolsB = plan["icolsA"], plan["icolsB"]
    sched = plan["sched"]

    nc = bacc.Bacc("TRN2", target_bir_lowering=False, debug=False,
                   num_devices=C, num_swdge_queues=4)

    xs_in = nc.dram_tensor("xs", [P, NCHT * F], BF16, kind="ExternalInput")
    Sm_in = nc.dram_tensor("Sm", [P, NCHT * P], BF16, kind="ExternalInput")
    idxA_in = nc.dram_tensor("idxA", [P, max(icolsA, 8)], I16, kind="ExternalInput")
    idxB_in = nc.dram_tensor("idxB", [P, max(icolsB, 8)], I16, kind="ExternalInput")
    Sp_in = nc.dram_tensor("Sp", [P, NB * G], BF16, kind="ExternalInput")
    W1_in = nc.dram_tensor("W1", [F, F], BF16, kind="ExternalInput")
    b1_in = nc.dram_tensor("b1r", [1, F], BF16, kind="ExternalInput")
    W2_in = nc.dram_tensor("W2", [F, F], BF16, kind="ExternalInput")
    b2_in = nc.dram_tensor("b2r", [1, F], BF16, kind="ExternalInput")
    Wfc_in = nc.dram_tensor("Wfc", [F, OUT], F32, kind="ExternalInput")
    bfc_in = nc.dram_tensor("bfcb", [P, OUT], F32, kind="ExternalInput")
    invc_in = nc.dram_tensor("invc", [F, G], F32, kind="ExternalInput")
    out = nc.dram_tensor("out", [G, OUT], F32, kind="ExternalOutput")

    gq = [0]

    with tile.TileContext(nc) as tc:
        with (
            tc.tile_pool(name="const", bufs=1) as cp,
            tc.tile_pool(name="stream", bufs=2) as stp,
            tc.tile_pool(name="smask", bufs=3) as smp,
            tc.tile_pool(name="gpool", bufs=3) as gp,
            tc.tile_pool(name="epool", bufs=4) as ep,
            tc.tile_pool(name="psA", bufs=3, space="PSUM") as psA,
            tc.tile_pool(name="psH", bufs=2, space="PSUM") as psH,
            tc.tile_pool(name="psP", bufs=1, space="PSUM") as psP,
            tc.tile_pool(name="dram", bufs=1, space="DRAM") as dram,
        ):
            # ---- constants ----
            idxA_sb = cp.tile([P, max(icolsA, 8)], I16, tag="idxA")
            nc.scalar.dma_start(idxA_sb[:], idxA_in[:])
            idxB_sb = cp.tile([P, max(icolsB, 8)], I16, tag="idxB")
            nc.scalar.dma_start(idxB_sb[:], idxB_in[:])
            Sp_sb = cp.tile([P, NB, G], BF16, tag="Sp")
            nc.scalar.dma_start(Sp_sb[:], Sp_in[:])
            W1_sb = cp.tile([F, F], BF16, tag="W1")
            nc.sync.dma_start(W1_sb[:], W1_in[:])
            b1_sb = cp.tile([1, F], BF16, tag="b1")
            nc.sync.dma_start(b1_sb[:], b1_in[:])
            W2_sb = cp.tile([F, F], BF16, tag="W2")
            nc.sync.dma_start(W2_sb[:], W2_in[:])
            b2_sb = cp.tile([1, F], BF16, tag="b2")
            nc.sync.dma_start(b2_sb[:], b2_in[:])
            Wfc_sb = cp.tile([F, OUT], F32, tag="Wfc")
            nc.sync.dma_start(Wfc_sb[:], Wfc_in[:])
            bfc_sb = cp.tile([P, OUT], F32, tag="bfc")
            nc.sync.dma_start(bfc_sb[:], bfc_in[:])
            invc_sb = cp.tile([F, G], F32, tag="invc")
            nc.sync.dma_start(invc_sb[:], invc_in[:])
            ones_sb = cp.tile([1, P], BF16, tag="ones")
            nc.vector.memset(ones_sb[:], 1.0)

            own_sb = cp.tile([P, NB, F], BF16, tag="own")

            shardA = dram.tile([SA_ROWS, F], BF16)
            shardB = dram.tile([SB_ROWS, F], BF16)
            fullA = dram.tile([C * SA_ROWS, F], BF16, addr_space="Shared")
            fullB = dram.tile([C * SB_ROWS, F], BF16, addr_space="Shared")
            h1pad = dram.tile([N, 2 * F], BF16)
            pool_in = dram.tile([F, G], F32)
            pool_out = dram.tile([F, G], F32, addr_space="Shared")

            pool_ps = psP.tile([F, G], F32, tag="pool")

            def epilogue(b, aggT, W_sb, brow_sb, layer):
                agg_sb = ep.tile([F, P], BF16, tag="agg_sb")
                nc.scalar.copy(agg_sb[:], aggT[:])
                h_ps = psH.tile([P, F], F32, tag="h")
                nc.tensor.matmul(h_ps[:], lhsT=agg_sb[:], rhs=W_sb[:],
                                 start=True, stop=False)
                nc.tensor.matmul(h_ps[:], lhsT=ones_sb[:], rhs=brow_sb[:],
                                 start=False, stop=True)
                if layer == 1:
                    nc.scalar.activation(own_sb[:, b, :], h_ps[:],
                                         mybir.ActivationFunctionType.Tanh)
                    if b < ABLOCKS:
                        r0 = b * P
                        nc.sync.dma_start(shardA[r0:r0 + P, :], own_sb[:, b, :])
                    else:
                        r0 = (b - ABLOCKS) * P
                        rows = min(P, SB_ROWS - r0)
                        nc.sync.dma_start(shardB[r0:r0 + rows, :],
                                          own_sb[:rows, b, :])
                else:
                    h2t = ep.tile([P, F], BF16, tag="h2t")
                    nc.scalar.activation(h2t[:], h_ps[:],
                                         mybir.ActivationFunctionType.Tanh)
                    nc.tensor.matmul(pool_ps[:], lhsT=h2t[:],
                                     rhs=Sp_sb[:, b, :],
                                     start=(b == 0), stop=(b == NB - 1),
                                     skip_group_check=True)

            # =================== Layer 1 (streamed) ===================
            ch_off = 0
            for s in range(NSET):
                ns = int(n_set[s])
                st = stp.tile([P, ns * F], BF16, tag="st")
                nc.scalar.dma_start(st[:], xs_in[:, ch_off * F:(ch_off + ns) * F])
                S_t = smp.tile([P, ns * P], BF16, tag="Sm")
                nc.sync.dma_start(S_t[:], Sm_in[:, ch_off * P:(ch_off + ns) * P])
                mms = sched[s]
                for b in _set_blocks(s):
                    kis = [k for k, (bb, kind, col) in enumerate(mms) if bb == b]
                    aggT = psA.tile([F, P], F32, tag="aggT")
                    for j, k in enumerate(kis):
                        nc.tensor.matmul(
                            aggT[:],
                            lhsT=st[:, k * F:(k + 1) * F],
                            rhs=S_t[:, k * P:(k + 1) * P],
                            start=(j == 0), stop=(j == len(kis) - 1),
                        )
                    epilogue(b, aggT, W1_sb, b1_sb, 1)
                    if b == ABLOCKS - 1:
                        nc.gpsimd.collective_compute(
                            "AllGather", mybir.AluOpType.bypass,
                            ins=[shardA.opt()], outs=[fullA.opt()],
                            replica_groups=[list(range(C))],
                        )
                ch_off += ns
                # spread the h1pad expand of the A half across later sets
                if 7 <= s <= 10:
                    for cc in range(2 * (s - 7), 2 * (s - 7) + 2):
                        nc.sync.dma_start(
                            h1pad[cc * NSH:cc * NSH + SA_ROWS, 0:F],
                            fullA[cc * SA_ROWS:(cc + 1) * SA_ROWS, :])

            nc.gpsimd.collective_compute(
                "AllGather", mybir.AluOpType.bypass,
                ins=[shardB.opt()], outs=[fullB.opt()],
                replica_groups=[list(range(C))],
            )
            for cc in range(C):
                nc.scalar.dma_start(
                    h1pad[cc * NSH + SA_ROWS:(cc + 1) * NSH, 0:F],
                    fullB[cc * SB_ROWS:(cc + 1) * SB_ROWS, :])

            # =================== Layer 2 (gathered) ===================
            acol = bcol = 0
            ch_off = 0
            for s in range(NSET):
                ns = int(n_set[s])
                nAs, nBs = int(nA_set[s]), int(nB_set[s])
                gtA = gp.tile([P, nAs, 2 * F], BF16, tag="gtA")
                for (c0, c1) in _pieces(nAs, 3):
                    q = gq[0] % 4
                    gq[0] += 1
                    nc.gpsimd.dma_gather(
                        gtA[:, c0:c1, :], h1pad[0:ASPLIT, :],
                        idxA_sb[:, acol + c0 * 8:acol + c1 * 8],
                        (c1 - c0) * P, (c1 - c0) * P, 2 * F,
                        single_packet=False, queue_num=q,
                    )
                gtB = gp.tile([P, nBs, 2 * F], BF16, tag="gtB")
                for (c0, c1) in _pieces(nBs, 2):
                    q = gq[0] % 4
                    gq[0] += 1
                    nc.gpsimd.dma_gather(
                        gtB[:, c0:c1, :], h1pad[ASPLIT:N, :],
                        idxB_sb[:, bcol + c0 * 8:bcol + c1 * 8],
                        (c1 - c0) * P, (c1 - c0) * P, 2 * F,
                        single_packet=False, queue_num=q,
                    )
                S_t = smp.tile([P, ns * P], BF16, tag="Sm")
                nc.sync.dma_start(S_t[:], Sm_in[:, ch_off * P:(ch_off + ns) * P])
                mms = sched[s]
                for b in _set_blocks(s):
                    kis = [k for k, (bb, kind, col) in enumerate(mms) if bb == b]
                    aggT = psA.tile([F, P], F32, tag="aggT")
                    for j, k in enumerate(kis):
                        _, kind, col = mms[k]
                        if kind == 0:
                            lhsT = own_sb[:, b, :]
                        elif kind == 1:
                            lhsT = gtA[:, col, 0:F]
                        else:
                            lhsT = gtB[:, col, 0:F]
                        nc.tensor.matmul(
                            aggT[:], lhsT=lhsT,
                            rhs=S_t[:, k * P:(k + 1) * P],
                            start=(j == 0), stop=(j == len(kis) - 1),
                        )
                    epilogue(b, aggT, W2_sb, b2_sb, 2)
                acol += nAs * (P // 16)
                bcol += nBs * (P // 16)
                ch_off += ns

            # ---- pooled tail ----
            poolT = ep.tile([F, G], F32, tag="poolT")
            nc.vector.tensor_copy(poolT[:], pool_ps[:])
            nc.sync.dma_start(pool_in[:], poolT[:])
            nc.gpsimd.collective_compute(
                "AllReduce", mybir.AluOpType.add,
                ins=[pool_in.opt()], outs=[pool_out.opt()],
                replica_groups=[list(range(C))],
            )
            poolR = ep.tile([F, G], F32, tag="poolR")
            nc.sync.dma_start(poolR[:], pool_out[:])
            nc.vector.tensor_mul(poolR[:], poolR[:], invc_sb[:])
            fc_ps = psP.tile([G, OUT], F32, tag="fc")
            nc.tensor.matmul(fc_ps[:], lhsT=poolR[:], rhs=Wfc_sb[:],
                             start=True, stop=True)
            out_sb = ep.tile([G, OUT], F32, tag="out_sb")
            nc.vector.tensor_add(out_sb[:], fc_ps[:], bfc_sb[:])
            nc.sync.dma_start(out[:], out_sb[:])

    nc.compile()
    return nc


def _in_maps(plan, per_core, invc, W1, b1, W2, b2, Wfc, bfc):
    com = dict(
        W1=np.asarray(W1, np.float32).astype(BF),
        b1r=np.asarray(b1, np.float32).reshape(1, F).astype(BF),
        W2=np.asarray(W2, np.float32).astype(BF),
        b2r=np.asarray(b2, np.float32).reshape(1, F).astype(BF),
        Wfc=np.ascontiguousarray(np.asarray(Wfc, np.float32)),
        bfcb=np.tile(np.asarray(bfc, np.float32), (P, 1)),
        invc=np.tile(invc, (F, 1)),
    )
    maps = []
    for c in range(C):
        m = dict(com)
        m.update(per_core[c])
        m["Sp"] = m["Sp"].reshape(P, NB * G)
        maps.append({k: np.ascontiguousarray(v) for k, v in m.items()})
    return maps


_RUN_KWARGS = {}


def kernel(x, src, dst, batch, W1, b1, W2, b2, Wfc, bfc):
    plan, per_core, invc = _preprocess(x, src, dst, batch)
    nc = _build(plan)
    maps = _in_maps(plan, per_core, invc, W1, b1, W2, b2, Wfc, bfc)
    res = bass_utils.run_bass_kernel_spmd(
        nc, maps, core_ids=list(range(C)), **_RUN_KWARGS
    )
    kernel.last_results = res
    return np.asarray(res.results[0]["out"], np.float32)
